# revision 73
# baseline (speedup 1.0000x reference)
"""LIF spike scan kernel for Trainium2, SPMD over 8 NeuronCores.

Problem: x [B=64, T=8, C=128, H=32, W=32] f32.  Per (b,c,h,w) pixel, scan
over T:  v = tau*u + x_t ; s_t = (v > 1) ; u = v*(v <= 1).  Output spikes
[B, T, C, H, W] f32 (bit-exact vs the f32 reference).

Design (all per core, pure batch-parallel across cores, no collectives):
- C=128 is the SBUF partition dim, H*W=1024 the per-op free dim.
- Per t-step, three ops on three engines:
    DVE   v   = (mem * tau) + x_t        scalar_tensor_tensor, in place
    ACT   s_t = Sign(v - 1) -> uint8     f32->u8 saturation turns the -1
                                         of Sign into 0, i.e. Heaviside
    DVE   mem = (v <= 1) * v             scalar_tensor_tensor hard reset
- The membrane lives in PSUM: its 8B/elem of traffic comes off the
  contended SBUF ports (SBUF bandwidth is the global ceiling here).
- Spikes are stored as uint8 (exact 0/1) and widened to f32 on the host:
  output HBM traffic drops 4x.
- x is pre-shuffled on the host so every load/store is a 2D fully
  contiguous DMA; loads are 1MB chunks, double-ended buffering hides them.
"""

import numpy as np

B, T, C, HW = 64, 8, 128, 32 * 32
N_CORES = 8
B_LOC = B // N_CORES
TAU = 0.5
THRESH = 1.0
OUT_DT = "uint8"

_cache = {}


def _build_nc():
    from concourse import bacc, mybir, tile

    op = mybir.AluOpType
    nc = bacc.Bacc(
        "TRN2", target_bir_lowering=False, debug=False, num_devices=N_CORES
    )
    out_dt = getattr(mybir.dt, OUT_DT)
    # x is pre-shuffled on the host to [b, h, c, (t_local hw)] — exactly the
    # SBUF tile layout — so every load is one 2D fully-contiguous transfer.
    # FD=1024 per op: measured faster per element than 2048 (1.15 vs 1.29
    # cyc/elem; SBUF contention grows superlinearly with op size here).
    GB = 1  # batch rows per scan group
    NG = B_LOC // GB
    TH = T // 2  # t-steps per half
    x_ext = nc.dram_tensor(
        "x", [NG * 2, C, TH * GB * HW], mybir.dt.float32, kind="ExternalInput"
    ).ap()
    # Output mirrors the SBUF layout; the host unshuffles to [b,t,c,h,w].
    out_ext = nc.dram_tensor(
        "out", [NG * 2, C, TH * GB * HW], out_dt, kind="ExternalOutput"
    ).ap()

    F = GB * HW  # columns per t-step in a group tile

    with tile.TileContext(nc) as tc:
        with tc.tile_pool(name="pool", bufs=2) as pool, tc.tile_pool(
            name="psum", bufs=2, space="PSUM"
        ) as ppool:
            neg_thresh = pool.tile([C, 1], mybir.dt.float32, tag="bias", bufs=1)
            nc.vector.memset(neg_thresh, -THRESH)
            for g in range(NG):
                # Membrane state lives in PSUM: its 8B/elem of read/write
                # traffic comes off the contended SBUF ports.
                mem = ppool.tile([C, F], mybir.dt.float32, tag="m")
                for h in range(2):
                    # x chunk [c, t_local, b2, f]
                    xc = pool.tile([C, TH * F], mybir.dt.float32, tag="x", bufs=6)
                    spk = pool.tile([C, TH * F], out_dt, tag="s", bufs=4)
                    # two 1MB loads per chunk: smoother staging, faster ramp
                    HF = TH * F // 2
                    nc.sync.dma_start(out=xc[:, :HF], in_=x_ext[g * 2 + h, :, :HF])
                    nc.sync.dma_start(out=xc[:, HF:], in_=x_ext[g * 2 + h, :, HF:])
                    for tl in range(TH):
                        t = h * TH + tl
                        v = xc[:, tl * F : (tl + 1) * F]
                        if t > 0:
                            # v = tau*mem + x_t   (in place into the x tile;
                            # one PSUM access per DVE op — DVE has a single
                            # PSUM port, so v itself must stay in SBUF)
                            nc.vector.scalar_tensor_tensor(
                                out=v, in0=mem, scalar=TAU, in1=v,
                                op0=op.mult, op1=op.add,
                            )
                        if tl % 2 == 1:
                            # Sign(v-1) in {-1,0,1}; f32->u8 writeback
                            # saturates the -1 to 0, i.e. Heaviside.  Each
                            # ACT op covers TWO t-slices so only every
                            # second madd needs a DVE->ACT semaphore inc.
                            nc.scalar.activation(
                                out=spk[:, (tl - 1) * F : (tl + 1) * F],
                                in_=xc[:, (tl - 1) * F : (tl + 1) * F],
                                func=mybir.ActivationFunctionType.Sign,
                                bias=neg_thresh,
                            )
                        if t < T - 1:
                            # mem = (v <= 1) * v   (hard reset)
                            nc.vector.scalar_tensor_tensor(
                                out=mem, in0=v, scalar=THRESH, in1=v,
                                op0=op.is_le, op1=op.mult,
                            )
                    nc.sync.dma_start(out=out_ext[g * 2 + h], in_=spk)
    nc.compile()
    return nc


def _run(x: np.ndarray, trace: bool = False, tmpdir=None):
    from concourse.bass_utils import run_bass_kernel_spmd

    if "nc" not in _cache:
        _cache["nc"] = _build_nc()
    nc = _cache["nc"]
    x = np.asarray(x)
    # Pre-shuffle to the kernel's SBUF tile layout:
    # x[b=g*GB+bl, t=h*TH+tl, c, f] -> x_shuf[core, g, h, c, tl, bl, f]
    GB = 1
    NG, TH = B_LOC // GB, T // 2
    x6 = x.reshape(N_CORES, NG, GB, 2, TH, C, HW).astype(np.float32, copy=False)
    x_shuf = np.ascontiguousarray(x6.transpose(0, 1, 3, 5, 4, 2, 6)).reshape(
        N_CORES, NG * 2, C, TH * GB * HW
    )
    in_maps = [{"x": x_shuf[i]} for i in range(N_CORES)]
    res = run_bass_kernel_spmd(
        nc, in_maps, core_ids=list(range(N_CORES)), trace=trace, tmpdir=tmpdir
    )
    _cache["last_results"] = res
    outs = [res.results[i]["out"] for i in range(N_CORES)]
    # [ncores, g, h, c, tl, bl, f] -> [ncores, g, bl, h, tl, c, f] = [B, T, C, HW]
    out = np.stack(outs, axis=0).reshape(N_CORES, NG, 2, C, TH, GB, HW)
    out = out.transpose(0, 1, 5, 2, 4, 3, 6).reshape(B, T, C, HW)
    if out.dtype != np.float32:
        out = out.astype(np.float32)
    return np.ascontiguousarray(out).reshape(B, T, C, 32, 32)


def kernel(x: np.ndarray) -> np.ndarray:
    return _run(x, trace=False)



# revision 74
# speedup vs baseline: 1.0052x; 1.0052x over previous
"""LIF spike scan kernel for Trainium2, SPMD over 8 NeuronCores.

Problem: x [B=64, T=8, C=128, H=32, W=32] f32.  Per (b,c,h,w) pixel, scan
over T:  v = tau*u + x_t ; s_t = (v > 1) ; u = v*(v <= 1).  Output spikes
[B, T, C, H, W] f32 (bit-exact vs the f32 reference).

Design (all per core, pure batch-parallel across cores, no collectives):
- C=128 is the SBUF partition dim, H*W=1024 the per-op free dim.
- Per t-step, three ops on three engines:
    DVE   v   = (mem * tau) + x_t        scalar_tensor_tensor, in place
    ACT   s_t = Sign(v - 1) -> uint8     f32->u8 saturation turns the -1
                                         of Sign into 0, i.e. Heaviside
    DVE   mem = (v <= 1) * v             scalar_tensor_tensor hard reset
- The membrane lives in PSUM: its 8B/elem of traffic comes off the
  contended SBUF ports (SBUF bandwidth is the global ceiling here).
- Spikes are stored as uint8 (exact 0/1) and widened to f32 on the host:
  output HBM traffic drops 4x.
- x is pre-shuffled on the host so every load/store is a 2D fully
  contiguous DMA; loads are 1MB chunks, double-ended buffering hides them.
"""

import numpy as np

B, T, C, HW = 64, 8, 128, 32 * 32
N_CORES = 8
B_LOC = B // N_CORES
TAU = 0.5
THRESH = 1.0
OUT_DT = "uint8"

_cache = {}


def _build_nc():
    from concourse import bacc, mybir, tile

    op = mybir.AluOpType
    nc = bacc.Bacc(
        "TRN2", target_bir_lowering=False, debug=False, num_devices=N_CORES
    )
    out_dt = getattr(mybir.dt, OUT_DT)
    # x is pre-shuffled on the host to [b, h, c, (t_local hw)] — exactly the
    # SBUF tile layout — so every load is one 2D fully-contiguous transfer.
    # FD=1024 per op: measured faster per element than 2048 (1.15 vs 1.29
    # cyc/elem; SBUF contention grows superlinearly with op size here).
    GB = 1  # batch rows per scan group
    NG = B_LOC // GB
    TH = T // 2  # t-steps per half
    x_ext = nc.dram_tensor(
        "x", [NG * 2, C, TH * GB * HW], mybir.dt.float32, kind="ExternalInput"
    ).ap()
    # Output mirrors the SBUF layout; the host unshuffles to [b,t,c,h,w].
    out_ext = nc.dram_tensor(
        "out", [NG * 2, C, TH * GB * HW], out_dt, kind="ExternalOutput"
    ).ap()

    F = GB * HW  # columns per t-step in a group tile

    with tile.TileContext(nc) as tc:
        with tc.tile_pool(name="pool", bufs=2) as pool, tc.tile_pool(
            name="psum", bufs=2, space="PSUM"
        ) as ppool:
            neg_thresh = pool.tile([C, 1], mybir.dt.float32, tag="bias", bufs=1)
            nc.vector.memset(neg_thresh, -THRESH)
            for g in range(NG):
                # Membrane state lives in PSUM: its 8B/elem of read/write
                # traffic comes off the contended SBUF ports.
                mem = ppool.tile([C, F], mybir.dt.float32, tag="m")
                for h in range(2):
                    # x chunk [c, t_local, b2, f]
                    xc = pool.tile([C, TH * F], mybir.dt.float32, tag="x", bufs=6)
                    spk = pool.tile([C, TH * F], out_dt, tag="s", bufs=4)
                    # two 1MB loads per chunk: smoother staging, faster ramp
                    HF = TH * F // 2
                    nc.sync.dma_start(out=xc[:, :HF], in_=x_ext[g * 2 + h, :, :HF])
                    nc.sync.dma_start(out=xc[:, HF:], in_=x_ext[g * 2 + h, :, HF:])
                    for tl in range(TH):
                        t = h * TH + tl
                        v = xc[:, tl * F : (tl + 1) * F]
                        s = spk[:, tl * F : (tl + 1) * F]
                        if t > 0:
                            # v = tau*mem + x_t   (in place into the x tile;
                            # one PSUM access per DVE op — DVE has a single
                            # PSUM port, so v itself must stay in SBUF)
                            nc.vector.scalar_tensor_tensor(
                                out=v, in0=mem, scalar=TAU, in1=v,
                                op0=op.mult, op1=op.add,
                            )
                        # Sign(v-1) in {-1,0,1}; f32->u8 writeback saturates
                        # the -1 to 0, giving the Heaviside directly.
                        nc.scalar.activation(
                            out=s, in_=v,
                            func=mybir.ActivationFunctionType.Sign,
                            bias=neg_thresh,
                        )
                        if t < T - 1:
                            # mem = (v <= 1) * v   (hard reset)
                            nc.vector.scalar_tensor_tensor(
                                out=mem, in0=v, scalar=THRESH, in1=v,
                                op0=op.is_le, op1=op.mult,
                            )
                    nc.sync.dma_start(out=out_ext[g * 2 + h], in_=spk)
    nc.compile()
    return nc


def _run(x: np.ndarray, trace: bool = False, tmpdir=None):
    from concourse.bass_utils import run_bass_kernel_spmd

    if "nc" not in _cache:
        _cache["nc"] = _build_nc()
    nc = _cache["nc"]
    x = np.asarray(x)
    # Pre-shuffle to the kernel's SBUF tile layout:
    # x[b=g*GB+bl, t=h*TH+tl, c, f] -> x_shuf[core, g, h, c, tl, bl, f]
    GB = 1
    NG, TH = B_LOC // GB, T // 2
    x6 = x.reshape(N_CORES, NG, GB, 2, TH, C, HW).astype(np.float32, copy=False)
    x_shuf = np.ascontiguousarray(x6.transpose(0, 1, 3, 5, 4, 2, 6)).reshape(
        N_CORES, NG * 2, C, TH * GB * HW
    )
    in_maps = [{"x": x_shuf[i]} for i in range(N_CORES)]
    res = run_bass_kernel_spmd(
        nc, in_maps, core_ids=list(range(N_CORES)), trace=trace, tmpdir=tmpdir
    )
    _cache["last_results"] = res
    outs = [res.results[i]["out"] for i in range(N_CORES)]
    # [ncores, g, h, c, tl, bl, f] -> [ncores, g, bl, h, tl, c, f] = [B, T, C, HW]
    out = np.stack(outs, axis=0).reshape(N_CORES, NG, 2, C, TH, GB, HW)
    out = out.transpose(0, 1, 5, 2, 4, 3, 6).reshape(B, T, C, HW)
    if out.dtype != np.float32:
        out = out.astype(np.float32)
    return np.ascontiguousarray(out).reshape(B, T, C, 32, 32)


def kernel(x: np.ndarray) -> np.ndarray:
    return _run(x, trace=False)

